# revision 1
# baseline (speedup 1.0000x reference)
"""GCNConv on 8 Trainium2 NeuronCores (Bass/Tile, SPMD).

out = D^-1/2 (A+I) D^-1/2 (X @ W.T),   deg = in-degree(col) + 1

Math refactoring (exact in real arithmetic):
    agg[r]  = sum_{e: dst=r} d[col_e] * X[col_e]      (self loop = edge (r,r))
    out[r]  = d[r] * (agg[r] @ W.T)                   (d = deg^-1/2)

Distribution: destinations (rows) are sharded across the 8 cores (12500
each); each core processes the edges whose destination lands in its shard.
X and W are replicated so any core can read any source row.

Device algorithm per core (one SPMD program; per-core index tables are
padded into a common, max-over-cores structure so SPMD is preserved):

  * Edge slots: edges (+ self loops) are grouped into segments by (range of
    RNG_DTS dest-tiles, source-chunk c of 25000 rows), sorted by destination
    inside each segment and packed densely (slot i of a gather lives at SBUF
    partition i%128, free block i//128).  Trailing pad entries use index -1
    and a per-core valid-count register, so padding costs zero descriptors.
  * Gather: one `dma_gather` (int16 indices relative to the chunk base) per
    segment pulls the 512-byte X rows of its slots.  This dominates the
    runtime and is descriptor-latency-bound (~9 ns/row aggregate), hence the
    dense packing: descriptors == real edges only.
  * Scale: one DVE multiply per segment applies d[col] per slot
    (per-partition scalar broadcast over the 128 features); d is computed on
    device from the integer degree counts (ACT sqrt + DVE reciprocal).
  * Segmented sum via PE: per 128-slot tile, a host-built 0/1 selection
    matrix S (slots x dests, bf16 in DRAM, cast to f32 during the DMA load)
    routes slots to destinations: psum[feat, dest] += g_tile.T @ S_tile,
    accumulating into a range-wide one-bank PSUM tile [128, RNG_DTS*128].
  * Finalize per dest-tile: copy PSUM->SBUF, matmul with W.T (contraction
    over features), scale by d[dest] (per-partition scalar), DMA out.

The host does index marshaling only (bucketing, sorting, degree counts, 0/1
selection structure); all floating-point math on X/W runs on device.
"""

import math

import numpy as np
import ml_dtypes

import concourse.bacc as bacc
import concourse.mybir as mybir
import concourse.tile as tile
from concourse.bass_utils import run_bass_kernel_spmd
from concourse import library_config

NCORES = 8
P = 128
CH_SPAN = 25000          # source rows per gather chunk (int16-indexable)
RNG_DTS = 4              # dest-tiles per range (psum tile = 1 bank = 512 f32)
DEG_PAD = 1.0e30         # pad degree -> d ~ 0

F32 = mybir.dt.float32
BF16 = mybir.dt.bfloat16
I16 = mybir.dt.int16


class Plan:
    pass


# ----------------------------------------------------------------------------
# Host-side index marshaling
# ----------------------------------------------------------------------------

def _preprocess(edge_index: np.ndarray, n_nodes: int):
    ns = n_nodes // NCORES
    rt = math.ceil(ns / P)
    nch = math.ceil(n_nodes / CH_SPAN)
    nrng = math.ceil(rt / RNG_DTS)

    row = np.asarray(edge_index[0]).astype(np.int64)
    col = np.asarray(edge_index[1]).astype(np.int64)
    deg = (np.bincount(col, minlength=n_nodes) + 1).astype(np.float32)

    core = row // ns
    cores = []
    for m in range(NCORES):
        sel = core == m
        r_l = row[sel] - m * ns
        c_g = col[sel]
        r_l = np.concatenate([r_l, np.arange(ns, dtype=np.int64)])
        c_g = np.concatenate([c_g, np.arange(ns, dtype=np.int64) + m * ns])
        rg = r_l // (RNG_DTS * P)
        ch = np.minimum(c_g // CH_SPAN, nch - 1)
        order = np.lexsort((c_g, r_l, ch, rg))
        r_l, c_g = r_l[order], c_g[order]
        code = rg[order] * nch + ch[order]
        bounds = np.searchsorted(code, np.arange(nrng * nch + 1))
        cores.append(dict(r_l=r_l, c_g=c_g, bounds=bounds))

    # segment tile counts: max over cores (packed, no per-dt padding)
    plan = Plan()
    plan.ns, plan.rt, plan.nch, plan.nrng = ns, rt, nch, nrng
    plan.segs = []
    jtot = 0
    for rg in range(nrng):
        for c in range(nch):
            g = rg * nch + c
            ntiles = 0
            for m in range(NCORES):
                b = cores[m]["bounds"]
                ntiles = max(ntiles, (int(b[g + 1] - b[g]) + P - 1) // P)
            if ntiles == 0:
                continue
            plan.segs.append(dict(base=c * CH_SPAN, t16_0=jtot * 8,
                                  n=ntiles * P, j0=jtot, ntiles=ntiles,
                                  rng=rg, c=c, g=g))
            jtot += ntiles
    plan.jtot = jtot
    plan.tot16 = jtot * 8

    nslots = jtot * P
    gidx = np.zeros((NCORES, P, plan.tot16), np.int16)
    deg_col = np.full((NCORES, nslots), DEG_PAD, np.float32)
    dest_arr = np.full((NCORES, nslots), -1, np.int64)  # rel to range base
    cnts = np.zeros((NCORES, max(1, len(plan.segs))), np.int32)
    for m in range(NCORES):
        r_l, c_g, b = cores[m]["r_l"], cores[m]["c_g"], cores[m]["bounds"]
        idx16 = np.full(nslots, -1, np.int16)
        for si, seg in enumerate(plan.segs):
            g = seg["g"]
            lo, hi = int(b[g]), int(b[g + 1])
            n = hi - lo
            if n == 0:
                # still need >= 1 valid index (dummy row 0, zero S row)
                idx16[seg["j0"] * P] = 0
                cnts[m, si] = 1
                continue
            cnts[m, si] = n
            s0 = seg["j0"] * P
            cg = c_g[lo:hi]
            idx16[s0:s0 + n] = (cg - seg["base"]).astype(np.int16)
            deg_col[m, s0:s0 + n] = deg[cg]
            dest_arr[m, s0:s0 + n] = (r_l[lo:hi]
                                      - seg["rng"] * RNG_DTS * P)
        w = idx16.reshape(plan.tot16, 16).T
        gidx[m] = np.tile(w, (8, 1))

    # common per-tile S frames (dmin/nd = union over cores, within the
    # RNG_DTS*128-wide range -> nd <= 512 always)
    da = dest_arr.reshape(NCORES, jtot, P)
    da_min = np.where(da < 0, 10 ** 9, da).min(axis=(0, 2))
    da_max = da.max(axis=(0, 2))
    plan.dmin = da_min.astype(np.int64)
    plan.nd = (da_max - da_min + 1).astype(np.int64)
    assert (plan.nd >= 1).all() and (plan.nd <= RNG_DTS * P).all()
    plan.s0 = np.zeros(jtot + 1, np.int64)
    np.cumsum(plan.nd, out=plan.s0[1:])
    plan.stot = int(plan.s0[-1])

    # S matrices
    s_pack = np.zeros((NCORES, P, plan.stot), ml_dtypes.bfloat16)
    ar = np.arange(P)
    for m in range(NCORES):
        for t in range(jtot):
            dl = da[m, t]
            v = dl >= 0
            if not v.any():
                continue
            blk = np.zeros((P, int(plan.nd[t])), np.float32)
            blk[ar[v], dl[v] - plan.dmin[t]] = 1.0
            s_pack[m, :, plan.s0[t]:plan.s0[t + 1]] = blk

    deg_nat = np.full((NCORES, P, rt), 1.0, np.float32)
    for m in range(NCORES):
        d = np.full(rt * P, 1.0, np.float32)
        d[:ns] = deg[m * ns:(m + 1) * ns]
        deg_nat[m] = d.reshape(rt, P).T

    deg_col = deg_col.reshape(NCORES, jtot, P).transpose(0, 2, 1)

    # per-range tile spans (S streamed per range)
    plan.rng_tiles = []
    seg_by_rng = {}
    for si, seg in enumerate(plan.segs):
        seg_by_rng.setdefault(seg["rng"], []).append(si)
    t = 0
    for rg in range(nrng):
        t0 = t
        for si in seg_by_rng.get(rg, []):
            t += plan.segs[si]["ntiles"]
        plan.rng_tiles.append((t0, t))
    plan.seg_by_rng = seg_by_rng
    plan.swmax = max((int(plan.s0[t1] - plan.s0[t0])
                      for t0, t1 in plan.rng_tiles if t1 > t0), default=1)
    plan.nmax = max(s["n"] for s in plan.segs)

    data = dict(gidx=gidx, deg_col=np.ascontiguousarray(deg_col),
                s_pack=s_pack, deg_nat=deg_nat, cnts=cnts)
    return plan, data


# ----------------------------------------------------------------------------
# Device program (identical for all cores)
# ----------------------------------------------------------------------------

def _build_nc(n_nodes: int, plan: Plan):
    ns, rt, nch, nrng = plan.ns, plan.rt, plan.nch, plan.nrng
    nc = bacc.Bacc("TRN2", target_bir_lowering=False, debug=False,
                   num_devices=NCORES)

    x_d = nc.dram_tensor("x", [n_nodes, P], F32, kind="ExternalInput").ap()
    wt_d = nc.dram_tensor("wt", [P, P], F32, kind="ExternalInput").ap()
    gix_d = nc.dram_tensor("gidx", [P, plan.tot16], I16,
                           kind="ExternalInput").ap()
    dcol_d = nc.dram_tensor("deg_col", [P, plan.jtot], F32,
                            kind="ExternalInput").ap()
    dnat_d = nc.dram_tensor("deg_nat", [P, rt], F32,
                            kind="ExternalInput").ap()
    s_d = nc.dram_tensor("s_pack", [P, plan.stot], BF16,
                         kind="ExternalInput").ap()
    cnt_d = nc.dram_tensor("cnts", [1, max(1, len(plan.segs))],
                           mybir.dt.int32, kind="ExternalInput").ap()
    out_d = nc.dram_tensor("out", [rt * P, P], F32, kind="ExternalOutput").ap()

    pw = RNG_DTS * P
    with tile.TileContext(nc) as tc:
        nc.gpsimd.load_library(library_config.mlp)
        with (
            tc.tile_pool(name="const", bufs=1) as cpool,
            tc.tile_pool(name="gbuf", bufs=3) as gpool,
            tc.tile_pool(name="sbuf_s", bufs=2) as spool,
            tc.tile_pool(name="fin", bufs=4) as fpool,
            tc.tile_pool(name="pacc", bufs=4, space="PSUM") as papool,
            tc.tile_pool(name="pout", bufs=2, space="PSUM") as popool,
        ):
            wt_sb = cpool.tile([P, P], F32)
            nc.sync.dma_start(out=wt_sb[:], in_=wt_d[:, :])
            gidx_sb = cpool.tile([P, plan.tot16], I16)
            nc.sync.dma_start(out=gidx_sb[:], in_=gix_d[:, :])

            dcol_sb = cpool.tile([P, plan.jtot], F32)
            nc.sync.dma_start(out=dcol_sb[:], in_=dcol_d[:, :])
            nc.scalar.activation(dcol_sb[:], dcol_sb[:],
                                 mybir.ActivationFunctionType.Sqrt)
            d_col = cpool.tile([P, plan.jtot], F32)
            nc.vector.reciprocal(d_col[:], dcol_sb[:])

            dnat_sb = cpool.tile([P, rt], F32)
            nc.sync.dma_start(out=dnat_sb[:], in_=dnat_d[:, :])
            nc.scalar.activation(dnat_sb[:], dnat_sb[:],
                                 mybir.ActivationFunctionType.Sqrt)
            d_nat = cpool.tile([P, rt], F32)
            nc.vector.reciprocal(d_nat[:], dnat_sb[:])

            zcol = cpool.tile([1, P], BF16)
            nc.vector.memset(zcol[:], 0.0)
            zrow = cpool.tile([1, pw], BF16)
            nc.vector.memset(zrow[:], 0.0)

            cnt_sb = cpool.tile([1, max(1, len(plan.segs))], mybir.dt.int32)
            nc.sync.dma_start(out=cnt_sb[:], in_=cnt_d[:, :])
            cnt_regs = [nc.gpsimd.alloc_register(f"cntr{i}") for i in range(4)]

            for rg in range(nrng):
                t0, t1 = plan.rng_tiles[rg]
                if t1 == t0:
                    continue
                sw0, sw1 = int(plan.s0[t0]), int(plan.s0[t1])
                s_sb = spool.tile([P, plan.swmax], F32, tag="s_sb")
                # bf16 -> f32 cast during DMA (SWDGE)
                nc.gpsimd.dma_start(out=s_sb[:, :sw1 - sw0],
                                    in_=s_d[:, sw0:sw1])

                pt = papool.tile([P, pw], F32, tag="pacc")
                nc.tensor.matmul(pt[:], lhsT=zcol[:], rhs=zrow[:],
                                 start=True, stop=False,
                                 skip_group_check=True)

                segs_rng = plan.seg_by_rng.get(rg, [])
                for k, si in enumerate(segs_rng):
                    seg = plan.segs[si]
                    jseg, nseg = seg["ntiles"], seg["n"]
                    g = gpool.tile([P, plan.nmax], F32, tag="g")
                    g3 = g[:, :nseg].rearrange("p (j f) -> p j f", f=P)
                    # pad slots are skipped by the gather (idx -1); zero them
                    # so the scale/matmuls see no stale garbage
                    nc.vector.memset(g[:, :nseg], 0.0)
                    span = min(CH_SPAN, n_nodes - seg["base"])
                    creg = cnt_regs[si % len(cnt_regs)]
                    nc.gpsimd.reg_load(creg, cnt_sb[0:1, si:si + 1])
                    nc.gpsimd.dma_gather(
                        g3, x_d[seg["base"]:seg["base"] + span, :],
                        gidx_sb[:, seg["t16_0"]:seg["t16_0"] + jseg * 8],
                        nseg, creg, P, single_packet=False,
                    )
                    dsl = d_col[:, seg["j0"]:seg["j0"] + jseg]
                    nc.vector.tensor_mul(
                        g3, g3, dsl[:, :, None].to_broadcast([P, jseg, P]))
                    for jj in range(jseg):
                        t = seg["j0"] + jj
                        dmin, nd = int(plan.dmin[t]), int(plan.nd[t])
                        sa = int(plan.s0[t]) - sw0
                        is_last = (k == len(segs_rng) - 1 and jj == jseg - 1)
                        nc.tensor.matmul(
                            pt[:, dmin:dmin + nd],
                            lhsT=g[:, jj * P:(jj + 1) * P],
                            rhs=s_sb[:, sa:sa + nd],
                            start=False, stop=is_last,
                            skip_group_check=True,
                        )

                for dl in range(min(RNG_DTS, rt - rg * RNG_DTS)):
                    dt = rg * RNG_DTS + dl
                    aggt = fpool.tile([P, P], F32, tag="aggt")
                    nc.vector.tensor_copy(aggt[:], pt[:, dl * P:(dl + 1) * P])
                    op = popool.tile([P, P], F32, tag="op")
                    nc.tensor.matmul(op[:], lhsT=aggt[:], rhs=wt_sb[:],
                                     start=True, stop=True)
                    ob = fpool.tile([P, P], F32, tag="ob")
                    nc.vector.tensor_scalar_mul(ob[:], op[:],
                                                d_nat[:, dt:dt + 1])
                    nc.sync.dma_start(out=out_d[dt * P:(dt + 1) * P, :],
                                      in_=ob[:])
    nc.compile()
    return nc


# ----------------------------------------------------------------------------
# Entry point
# ----------------------------------------------------------------------------

_CACHE: dict = {}


def _prepare(X, W, edge_index):
    X = np.ascontiguousarray(np.asarray(X, dtype=np.float32))
    W = np.asarray(W, dtype=np.float32)
    edge_index = np.asarray(edge_index)
    n = X.shape[0]
    plan, data = _preprocess(edge_index, n)
    key = (n, plan.jtot, plan.stot, tuple(s["n"] for s in plan.segs))
    if key not in _CACHE:
        _CACHE.clear()
        _CACHE[key] = _build_nc(n, plan)
    nc = _CACHE[key]
    wt = np.ascontiguousarray(W.T)
    in_maps = [
        {
            "x": X,
            "wt": wt,
            "gidx": np.ascontiguousarray(data["gidx"][m]),
            "deg_col": np.ascontiguousarray(data["deg_col"][m]),
            "deg_nat": np.ascontiguousarray(data["deg_nat"][m]),
            "s_pack": np.ascontiguousarray(data["s_pack"][m]),
            "cnts": np.ascontiguousarray(data["cnts"][m][None, :]),
        }
        for m in range(NCORES)
    ]
    return nc, in_maps, plan


def kernel(X, W, edge_index):
    nc, in_maps, plan = _prepare(X, W, edge_index)
    res = run_bass_kernel_spmd(nc, in_maps, core_ids=list(range(NCORES)))
    ns = plan.ns
    return np.concatenate([res.results[m]["out"][:ns] for m in range(NCORES)],
                          axis=0)



# revision 2
# speedup vs baseline: 3.2918x; 3.2918x over previous
"""GCNConv on 8 Trainium2 NeuronCores (Bass/Tile, SPMD).

out = D^-1/2 (A+I) D^-1/2 (X @ W.T),   deg = in-degree(col) + 1

Math refactoring (exact in real arithmetic):
    agg[r] = sum_{e: dst=r} (d[col_e] * d[r]) * X[col_e]   (self loop = (r,r))
    out[r] = agg[r] @ W.T                                  (d = deg^-1/2)

Distribution: destinations (rows) sharded across 8 cores (12500 each); each
core processes the edges whose destination lands in its shard.  X and W are
replicated (staged bf16) so any core can read any source row.

Device algorithm per core (one SPMD program; per-core index tables are
padded into a common, max-over-cores structure so SPMD is preserved):

  * Edge slots: edges are grouped into segments by (range of 8 dest-tiles,
    source-chunk of 25000 rows), sorted by destination inside each segment
    and packed densely (slot i of a gather lives at SBUF partition i%128,
    free block i//128).  Trailing pad entries use index -1 and a per-core
    valid-count register, so padding costs zero descriptors.
  * Self loops get their own perfectly-balanced segment per range and are
    loaded with a strided HWDGE DMA from a per-core staged copy of the
    shard (no gather descriptors).
  * Gather: one `dma_gather` (int16 indices relative to the chunk base) per
    segment pulls the 256-byte bf16 X rows of its slots.  Gather descriptor
    generation (~6-9 ns/row, Q7-bound) dominates the runtime; it is spread
    over the 4 SWDGE queues (queue_num = segment % 4) and descriptors are
    emitted only for real edges.
  * Segmented sum via PE: per 128-slot tile, a host-built selection matrix
    S (slots x dests, bf16, entries d[col]*d[dest]) routes and scales slots
    to destinations: psum[feat, dest] += g_tile.T @ S_tile.  Ranges span
    1024 dests accumulated in two one-bank PSUM tiles; tiles whose dest
    window straddles the 512 boundary emit two matmuls.
  * Finalize per dest-tile: copy PSUM->SBUF (bf16), matmul with W.T
    (contraction over features), DMA out f32.

The host does index marshaling only (bucketing, sorting, degree-derived
edge values, selection structure); all math on X/W runs on device.
"""

import math

import numpy as np
import ml_dtypes

import concourse.bacc as bacc
import concourse.mybir as mybir
import concourse.tile as tile
from concourse.bass_utils import run_bass_kernel_spmd
from concourse import library_config

NCORES = 8
P = 128
CH_SPAN = 25000          # source rows per gather chunk (int16-indexable)
RDTS = 8                 # dest-tiles per range (2 psum banks of 4 each)
QUEUES = 4               # SWDGE queues (ucode max)
GBUFS = 6                # gather tile pool depth

F32 = mybir.dt.float32
BF16 = mybir.dt.bfloat16
I16 = mybir.dt.int16


class Plan:
    pass


# ----------------------------------------------------------------------------
# Host-side index marshaling
# ----------------------------------------------------------------------------

def _preprocess(edge_index: np.ndarray, n_nodes: int):
    ns = n_nodes // NCORES
    rt = math.ceil(ns / P)
    nch = math.ceil(n_nodes / CH_SPAN)
    nrng = math.ceil(rt / RDTS)
    nseg_ch = nch + 1        # extra pseudo-chunk: self loops

    row = np.asarray(edge_index[0]).astype(np.int64)
    col = np.asarray(edge_index[1]).astype(np.int64)
    deg = (np.bincount(col, minlength=n_nodes) + 1).astype(np.float32)
    d_inv = deg.astype(np.float64) ** -0.5

    core = row // ns
    cores = []
    for m in range(NCORES):
        sel = core == m
        r_l = row[sel] - m * ns
        c_g = col[sel]
        n_real = len(r_l)
        r_l = np.concatenate([r_l, np.arange(ns, dtype=np.int64)])
        c_g = np.concatenate([c_g, np.arange(ns, dtype=np.int64) + m * ns])
        rg = r_l // (RDTS * P)
        ch = np.minimum(c_g // CH_SPAN, nch - 1)
        ch[n_real:] = nch    # self loops -> own segment per range
        order = np.lexsort((c_g, r_l, ch, rg))
        r_l, c_g = r_l[order], c_g[order]
        code = rg[order] * nseg_ch + ch[order]
        bounds = np.searchsorted(code, np.arange(nrng * nseg_ch + 1))
        cores.append(dict(r_l=r_l, c_g=c_g, bounds=bounds))

    # segment tile counts: max over cores (packed, no per-dt padding)
    plan = Plan()
    plan.ns, plan.rt, plan.nch, plan.nrng = ns, rt, nch, nrng
    plan.segs = []
    jtot = 0
    for rg in range(nrng):
        for c in range(nseg_ch):
            g = rg * nseg_ch + c
            ntiles = 0
            for m in range(NCORES):
                b = cores[m]["bounds"]
                ntiles = max(ntiles, (int(b[g + 1] - b[g]) + P - 1) // P)
            if ntiles == 0:
                continue
            plan.segs.append(dict(base=min(c, nch - 1) * CH_SPAN,
                                  t16_0=jtot * 8,
                                  n=ntiles * P, j0=jtot, ntiles=ntiles,
                                  rng=rg, c=c, g=g, self_=(c == nch)))
            jtot += ntiles
    plan.jtot = jtot
    plan.tot16 = jtot * 8

    nslots = jtot * P
    gidx = np.zeros((NCORES, P, plan.tot16), np.int16)
    val_slot = np.zeros((NCORES, nslots), np.float32)  # d[col]*d[dest]
    dest_arr = np.full((NCORES, nslots), -1, np.int64)  # rel to range base
    cnts = np.zeros((NCORES, max(1, len(plan.segs))), np.int32)
    for m in range(NCORES):
        r_l, c_g, b = cores[m]["r_l"], cores[m]["c_g"], cores[m]["bounds"]
        idx16 = np.full(nslots, -1, np.int16)
        for si, seg in enumerate(plan.segs):
            g = seg["g"]
            lo, hi = int(b[g]), int(b[g + 1])
            n = hi - lo
            if n == 0:
                # still need >= 1 valid index (dummy row 0, zero S row)
                idx16[seg["j0"] * P] = 0
                cnts[m, si] = 1
                continue
            cnts[m, si] = n
            s0 = seg["j0"] * P
            cg = c_g[lo:hi]
            idx16[s0:s0 + n] = (cg - seg["base"]).astype(np.int16)
            val_slot[m, s0:s0 + n] = (d_inv[cg]
                                      * d_inv[m * ns + r_l[lo:hi]])
            dest_arr[m, s0:s0 + n] = (r_l[lo:hi]
                                      - seg["rng"] * RDTS * P)
        w = idx16.reshape(plan.tot16, 16).T
        gidx[m] = np.tile(w, (8, 1))

    # common per-tile S frames (dmin/nd = union over cores, within the
    # RDTS*128-wide range)
    da = dest_arr.reshape(NCORES, jtot, P)
    da_min = np.where(da < 0, 10 ** 9, da).min(axis=(0, 2))
    da_max = da.max(axis=(0, 2))
    plan.dmin = da_min.astype(np.int64)
    plan.nd = (da_max - da_min + 1).astype(np.int64)
    assert (plan.nd >= 1).all() and (plan.nd <= RDTS * P).all()
    plan.s0 = np.zeros(jtot + 1, np.int64)
    np.cumsum(plan.nd, out=plan.s0[1:])
    plan.stot = int(plan.s0[-1])

    # S matrices: one-hot scaled by the edge value d[col]*d[dest]
    vs = val_slot.reshape(NCORES, jtot, P)
    s_pack = np.zeros((NCORES, P, plan.stot), ml_dtypes.bfloat16)
    ar = np.arange(P)
    for m in range(NCORES):
        for t in range(jtot):
            dl = da[m, t]
            v = dl >= 0
            if not v.any():
                continue
            blk = np.zeros((P, int(plan.nd[t])), np.float32)
            blk[ar[v], dl[v] - plan.dmin[t]] = vs[m, t][v]
            s_pack[m, :, plan.s0[t]:plan.s0[t + 1]] = blk

    # per-range tile spans (S streamed per range)
    plan.rng_tiles = []
    seg_by_rng = {}
    for si, seg in enumerate(plan.segs):
        seg_by_rng.setdefault(seg["rng"], []).append(si)
    t = 0
    for rg in range(nrng):
        t0 = t
        for si in seg_by_rng.get(rg, []):
            t += plan.segs[si]["ntiles"]
        plan.rng_tiles.append((t0, t))
    plan.seg_by_rng = seg_by_rng
    plan.swmax = max((int(plan.s0[t1] - plan.s0[t0])
                      for t0, t1 in plan.rng_tiles if t1 > t0), default=1)
    plan.nmax = max(s["n"] for s in plan.segs)

    data = dict(gidx=gidx, s_pack=s_pack, cnts=cnts)
    return plan, data


# ----------------------------------------------------------------------------
# Device program (identical for all cores)
# ----------------------------------------------------------------------------

def _build_nc(n_nodes: int, plan: Plan):
    ns, rt, nch, nrng = plan.ns, plan.rt, plan.nch, plan.nrng
    nc = bacc.Bacc("TRN2", target_bir_lowering=False, debug=False,
                   num_devices=NCORES, num_swdge_queues=QUEUES)

    x_d = nc.dram_tensor("x", [n_nodes, P], BF16, kind="ExternalInput").ap()
    wt_d = nc.dram_tensor("wt", [P, P], BF16, kind="ExternalInput").ap()
    gix_d = nc.dram_tensor("gidx", [P, plan.tot16], I16,
                           kind="ExternalInput").ap()
    s_d = nc.dram_tensor("s_pack", [P, plan.stot], BF16,
                         kind="ExternalInput").ap()
    cnt_d = nc.dram_tensor("cnts", [1, max(1, len(plan.segs))],
                           mybir.dt.int32, kind="ExternalInput").ap()
    xs_d = nc.dram_tensor("xself", [rt * P, P], BF16,
                          kind="ExternalInput").ap()
    out_d = nc.dram_tensor("out", [rt * P, P], F32, kind="ExternalOutput").ap()

    with tile.TileContext(nc) as tc:
        nc.gpsimd.load_library(library_config.mlp)
        with (
            tc.tile_pool(name="const", bufs=1) as cpool,
            tc.tile_pool(name="gbuf", bufs=GBUFS) as gpool,
            tc.tile_pool(name="sbuf_s", bufs=2) as spool,
            tc.tile_pool(name="fin", bufs=4) as fpool,
            tc.tile_pool(name="pacc", bufs=2, space="PSUM") as papool,
            tc.tile_pool(name="pout", bufs=2, space="PSUM") as popool,
        ):
            wt_sb = cpool.tile([P, P], BF16)
            nc.sync.dma_start(out=wt_sb[:], in_=wt_d[:, :])
            gidx_sb = cpool.tile([P, plan.tot16], I16)
            nc.sync.dma_start(out=gidx_sb[:], in_=gix_d[:, :])

            zcol = cpool.tile([1, P], BF16)
            nc.vector.memset(zcol[:], 0.0)
            zrow = cpool.tile([1, 4 * P], BF16)
            nc.vector.memset(zrow[:], 0.0)

            cnt_sb = cpool.tile([1, max(1, len(plan.segs))], mybir.dt.int32)
            nc.sync.dma_start(out=cnt_sb[:], in_=cnt_d[:, :])
            cnt_regs = [nc.gpsimd.alloc_register(f"cntr{i}") for i in range(4)]

            seg_no = 0
            for rg in range(nrng):
                t0, t1 = plan.rng_tiles[rg]
                if t1 == t0:
                    continue
                sw0, sw1 = int(plan.s0[t0]), int(plan.s0[t1])
                s_sb = spool.tile([P, plan.swmax], BF16, tag="s_sb")
                nc.sync.dma_start(out=s_sb[:, :sw1 - sw0],
                                  in_=s_d[:, sw0:sw1])

                # psum accumulators: one 1-bank [P, 512] tile per 4
                # dest-tiles (psum tensors must not exceed one bank)
                nhalf = (min(RDTS, rt - rg * RDTS) + 3) // 4
                pts = []
                for h in range(nhalf):
                    pth = papool.tile([P, 4 * P], F32, tag=f"pacc{h}")
                    nc.tensor.matmul(pth[:], lhsT=zcol[:], rhs=zrow[:],
                                     start=True, stop=False,
                                     skip_group_check=True)
                    pts.append(pth)

                segs_rng = plan.seg_by_rng.get(rg, [])
                # matmul pieces: split any tile window straddling col 512
                pieces = []
                for k, si in enumerate(segs_rng):
                    seg = plan.segs[si]
                    for jj in range(seg["ntiles"]):
                        t = seg["j0"] + jj
                        dmin, nd = int(plan.dmin[t]), int(plan.nd[t])
                        h0, h1 = dmin // 512, (dmin + nd - 1) // 512
                        for h in range(h0, h1 + 1):
                            c0 = max(dmin, h * 512)
                            c1 = min(dmin + nd, (h + 1) * 512)
                            pieces.append((k, jj, h, c0, c1))
                last_piece = {}
                for idx, (k, jj, h, c0, c1) in enumerate(pieces):
                    last_piece[h] = idx

                for k, si in enumerate(segs_rng):
                    seg = plan.segs[si]
                    jseg, nseg = seg["ntiles"], seg["n"]
                    g = gpool.tile([P, plan.nmax], BF16, tag="g")
                    g3 = g[:, :nseg].rearrange("p (j f) -> p j f", f=P)
                    # pad slots (gather idx -1) read stale data; S rows are
                    # zero there, so only the first touch of each pool buffer
                    # needs a memset (avoids inf/nan garbage at startup)
                    if seg_no < GBUFS:
                        nc.vector.memset(g[:], 0.0)
                    seg_no += 1
                    if seg.get("self_"):
                        r0 = rg * RDTS * P
                        xs_view = xs_d[r0:r0 + jseg * P, :].rearrange(
                            "(j p) f -> p j f", p=P)
                        nc.sync.dma_start(out=g3, in_=xs_view)
                    else:
                        span = min(CH_SPAN, n_nodes - seg["base"])
                        creg = cnt_regs[si % len(cnt_regs)]
                        nc.gpsimd.reg_load(creg, cnt_sb[0:1, si:si + 1])
                        nc.gpsimd.dma_gather(
                            g3, x_d[seg["base"]:seg["base"] + span, :],
                            gidx_sb[:, seg["t16_0"]:seg["t16_0"] + jseg * 8],
                            nseg, creg, P, single_packet=False,
                            queue_num=si % QUEUES,
                        )
                    for pidx, (pk, jj, h, c0, c1) in enumerate(pieces):
                        if pk != k:
                            continue
                        t = seg["j0"] + jj
                        dmin = int(plan.dmin[t])
                        sa = int(plan.s0[t]) - sw0 + (c0 - dmin)
                        nc.tensor.matmul(
                            pts[h][:, c0 - h * 512:c1 - h * 512],
                            lhsT=g[:, jj * P:(jj + 1) * P],
                            rhs=s_sb[:, sa:sa + (c1 - c0)],
                            start=False, stop=last_piece[h] == pidx,
                            skip_group_check=True,
                        )

                for dl in range(min(RDTS, rt - rg * RDTS)):
                    dt = rg * RDTS + dl
                    pt = pts[dl // 4]
                    dlh = dl % 4
                    aggt = fpool.tile([P, P], BF16, tag="aggt")
                    nc.vector.tensor_copy(aggt[:],
                                          pt[:, dlh * P:(dlh + 1) * P])
                    op = popool.tile([P, P], F32, tag="op")
                    nc.tensor.matmul(op[:], lhsT=aggt[:], rhs=wt_sb[:],
                                     start=True, stop=True)
                    ob = fpool.tile([P, P], F32, tag="ob")
                    nc.vector.tensor_copy(ob[:], op[:])
                    nc.sync.dma_start(out=out_d[dt * P:(dt + 1) * P, :],
                                      in_=ob[:])
    nc.compile()
    return nc


# ----------------------------------------------------------------------------
# Entry point
# ----------------------------------------------------------------------------

_CACHE: dict = {}


def _prepare(X, W, edge_index):
    X = np.ascontiguousarray(np.asarray(X, dtype=np.float32))
    W = np.asarray(W, dtype=np.float32)
    edge_index = np.asarray(edge_index)
    n = X.shape[0]
    plan, data = _preprocess(edge_index, n)
    key = (n, plan.jtot, plan.stot, tuple(s["n"] for s in plan.segs))
    if key not in _CACHE:
        _CACHE.clear()
        _CACHE[key] = _build_nc(n, plan)
    nc = _CACHE[key]
    xb = np.ascontiguousarray(X.astype(ml_dtypes.bfloat16))
    wtb = np.ascontiguousarray(W.T.astype(ml_dtypes.bfloat16))
    ns, rtp = plan.ns, plan.rt * P
    in_maps = []
    for m in range(NCORES):
        xs = np.zeros((rtp, P), ml_dtypes.bfloat16)
        xs[:ns] = xb[m * ns:(m + 1) * ns]
        in_maps.append({
            "x": xb,
            "wt": wtb,
            "gidx": np.ascontiguousarray(data["gidx"][m]),
            "s_pack": np.ascontiguousarray(data["s_pack"][m]),
            "cnts": np.ascontiguousarray(data["cnts"][m][None, :]),
            "xself": xs,
        })
    return nc, in_maps, plan


def kernel(X, W, edge_index):
    nc, in_maps, plan = _prepare(X, W, edge_index)
    res = run_bass_kernel_spmd(nc, in_maps, core_ids=list(range(NCORES)))
    ns = plan.ns
    return np.concatenate([res.results[m]["out"][:ns] for m in range(NCORES)],
                          axis=0)
